# revision 6
# baseline (speedup 1.0000x reference)
"""Trainium2 Bass kernel for ConformalMIL forward pass.

Self-contained: host-side param prep (numpy) + 8-core SPMD Bass/Tile kernel.

Math (per batch b):
  K = mexhat(w1)+mexhat(w2)+mexhat(w3)               [D, 19] (host)
  posT = depthwise_conv_S(xT, K)                     [D, S]  (DVE/ACT+GP taps)
  yT   = xT + Wp1^T posT  (+bp1 via host-folded bias) [D, S]  (PE, PSUM accum)
  LN stats per s: m, r=1/sqrt(var+eps) (PE ones-matmul + ACT ln/exp rows)
  LN is folded into attention:
    scores = r*(Wqk^T yT - m*qwsum) + qb - log(S);  attn = sigmoid(scores)
    v = r*(yT^T Wvg - m 1 wsum_v^T) + 1 bv^T
    o = attn @ v  computed as  attnR^T v_raw - (attnR^T m)*wsum_v + asum*bv
        with attnR = attn*r  (all rank-1 fixups on [32,512] tiles)
  head: o @ Wproj + bproj -> LN2 (same trick, tiny) -> relu(.@Wc1+bc1) @ Wc2 + bc2
"""

import sys, math, os
sys.path.insert(0, "/opt/trn_rl_repo")

import numpy as np
from contextlib import ExitStack

import concourse.bass as bass
import concourse.tile as tile
from concourse import bacc, mybir
from concourse.bass_utils import run_bass_kernel_spmd
from concourse.masks import make_identity

F32 = mybir.dt.float32
AF = mybir.ActivationFunctionType
OP = mybir.AluOpType

B, S, D, NCLS, H = 32, 2048, 512, 2, 16
HD = D // H
KSIZE = 19
HALF = (KSIZE - 1) // 2
N_CORES = 8
BC = B // N_CORES          # batches per core
SB = 512                   # s-block (PSUM bank width in fp32)
NBLK = S // SB
NDT = D // 128             # 128-partition d tiles
NST = S // 128             # 128-row s tiles
HN = H * NCLS              # 32 score rows, (h, n) h-major

# ---- tuning knobs -------------------------------------------------------
# engine per conv tap: 'dve' (scalar_tensor_tensor chain), 'actgp' (ACT
# scaled copy + GPSIMD add chain), 'pe' (fused diag(K_j)@Wp1 matmul pass)
TAP_ENGINE = ["dve"] * KSIZE
V_DRAIN = "act"            # psum->sbuf drain engine for v tiles: 'act'|'dve'
Y_DRAIN = "act"


def _mexhat_np(wave):
    scale, shift = wave[0], wave[1]                     # [D,1]
    xs = np.linspace(-HALF, HALF, KSIZE, dtype=np.float32)
    xg = xs[None, :] - shift                            # [D,19]
    C = 2.0 / (3.0 ** 0.5 * np.pi ** 0.25)
    u = xg / scale
    return (C * (1.0 - u ** 2) * np.exp(-0.5 * u ** 2) /
            np.sqrt(np.abs(scale))).astype(np.float32)


def host_consts(inp):
    """All parameter preprocessing, returns dict of np arrays for DMA."""
    c = {}
    K = (_mexhat_np(inp["wave1"]) + _mexhat_np(inp["wave2"])
         + _mexhat_np(inp["wave3"]))                    # [D,19]
    c["kk"] = np.ascontiguousarray(K)
    c["wp1"] = np.ascontiguousarray(inp["Wp1"])         # [D,D] lhsT natural
    # bp1 folded: y = x + pos@Wp1 + bp1 ; bp1 enters y rows (T layout bias
    # per e-partition). Handled as extra rank-1? bp1 is zeros in this
    # problem; keep exactness via e-partition bias added on the yT drain.
    c["bp1"] = np.ascontiguousarray(inp["bp1"].reshape(D, 1))
    g1 = inp["ln1_g"]; b1 = inp["ln1_b"]
    Wk = inp["Wkv"][:, :D]; Wv = inp["Wkv"][:, D:]
    Wkg = g1[:, None] * Wk                              # [D, D(c_k)]
    Wvg = np.ascontiguousarray(g1[:, None] * Wv)        # [D, D(c_v)]
    bk = b1 @ Wk                                        # [D]
    bv = b1 @ Wv                                        # [D]
    cls = inp["cls_token"][0]                           # [NCLS, D]
    q = (cls @ inp["Wq"]).reshape(NCLS, H, HD) * (HD ** -0.5)
    # qblk[(c_k),(h,n)] block diag; Wqk = Wkg @ qblk  [D, HN]
    Wqk = np.zeros((D, HN), np.float32)
    qw = np.zeros((HN,), np.float32)
    qb = np.zeros((HN,), np.float32)
    for h in range(H):
        for n in range(NCLS):
            col = h * NCLS + n
            Wqk[:, col] = Wkg[:, h * HD:(h + 1) * HD] @ q[n, h]
            qw[col] = Wqk[:, col].sum()
            qb[col] = q[n, h] @ bk[h * HD:(h + 1) * HD]
    c["wqk"] = Wqk
    c["qw_neg"] = (-qw).reshape(HN, 1)
    c["qb_ls"] = (qb - math.log(S)).reshape(HN, 1)
    c["wvg"] = Wvg
    c["wsumv_b32"] = np.tile(Wvg.sum(0)[None, :], (HN, 1)).astype(np.float32)
    c["bv_b32"] = np.tile(bv[None, :], (HN, 1)).astype(np.float32)
    # head
    c["wproj"] = np.ascontiguousarray(inp["Wproj"])     # [D(c), D(e)]
    c["bproj"] = np.ascontiguousarray(inp["bproj"].reshape(D, 1))
    g2 = inp["ln2_g"]; b2 = inp["ln2_b"]
    c["wc1"] = np.ascontiguousarray(g2[:, None] * inp["Wc1"])
    c["bc1"] = np.ascontiguousarray((inp["bc1"] + b2 @ inp["Wc1"]).reshape(D, 1))
    c["wc2"] = np.ascontiguousarray(inp["Wc2"])         # [D,1]
    c["bc2"] = float(inp["bc2"][0])
    # per-tap fused weights for 'pe' taps
    for j, eng in enumerate(TAP_ENGINE):
        if eng == "pe":
            c[f"wj{j}"] = np.ascontiguousarray(K[:, j:j + 1] * inp["Wp1"])
    return c


def build(bc=BC, s=S, tap_engine=None, bc2_val=0.0):
    """Build the SPMD Bass program for one core ([bc, s, D] shard)."""
    tap_engine = tap_engine or TAP_ENGINE
    nblk = s // SB
    nst = s // 128
    nc = bacc.Bacc("TRN2", target_bir_lowering=False, debug=False,
                   num_devices=N_CORES)

    x_d = nc.dram_tensor("x", [bc, s, D], F32, kind="ExternalInput").ap()
    kk_d = nc.dram_tensor("kk", [D, KSIZE], F32, kind="ExternalInput").ap()
    wp1_d = nc.dram_tensor("wp1", [D, D], F32, kind="ExternalInput").ap()
    bp1_d = nc.dram_tensor("bp1", [D, 1], F32, kind="ExternalInput").ap()
    wqk_d = nc.dram_tensor("wqk", [D, HN], F32, kind="ExternalInput").ap()
    qwn_d = nc.dram_tensor("qw_neg", [HN, 1], F32, kind="ExternalInput").ap()
    qbl_d = nc.dram_tensor("qb_ls", [HN, 1], F32, kind="ExternalInput").ap()
    wvg_d = nc.dram_tensor("wvg", [D, D], F32, kind="ExternalInput").ap()
    wsv_d = nc.dram_tensor("wsumv_b32", [HN, D], F32, kind="ExternalInput").ap()
    bv_d = nc.dram_tensor("bv_b32", [HN, D], F32, kind="ExternalInput").ap()
    wpr_d = nc.dram_tensor("wproj", [D, D], F32, kind="ExternalInput").ap()
    bpr_d = nc.dram_tensor("bproj", [D, 1], F32, kind="ExternalInput").ap()
    wc1_d = nc.dram_tensor("wc1", [D, D], F32, kind="ExternalInput").ap()
    bc1_d = nc.dram_tensor("bc1", [D, 1], F32, kind="ExternalInput").ap()
    wc2_d = nc.dram_tensor("wc2", [D, 1], F32, kind="ExternalInput").ap()
    wj_d = {j: nc.dram_tensor(f"wj{j}", [D, D], F32, kind="ExternalInput").ap()
            for j, e in enumerate(tap_engine) if e == "pe"}

    attn_d = nc.dram_tensor("attn", [bc, HN, s], F32, kind="ExternalOutput").ap()
    log_d = nc.dram_tensor("logits", [1, bc * NCLS], F32,
                           kind="ExternalOutput").ap()

    dve_taps = [j for j, e in enumerate(tap_engine) if e == "dve"]
    ag_taps = [j for j, e in enumerate(tap_engine) if e == "actgp"]
    pe_taps = [j for j, e in enumerate(tap_engine) if e == "pe"]
    assert dve_taps, "need at least one dve tap (chain head)"

    with tile.TileContext(nc) as tc, ExitStack() as ctx:
        # NB: bufs is per-TAG slot count; keep small, tags are explicit
        cs = ctx.enter_context(tc.tile_pool(name="consts", bufs=1))
        xin = ctx.enter_context(tc.tile_pool(name="xin", bufs=6))
        xtp = ctx.enter_context(tc.tile_pool(name="xtp", bufs=1))
        accp = ctx.enter_context(tc.tile_pool(name="accp", bufs=2))
        ydr = ctx.enter_context(tc.tile_pool(name="ydr", bufs=2))
        ysq = ctx.enter_context(tc.tile_pool(name="ysq", bufs=1))
        vdr = ctx.enter_context(tc.tile_pool(name="vdr", bufs=2))
        rows = ctx.enter_context(tc.tile_pool(name="rows", bufs=6))
        sm32 = ctx.enter_context(tc.tile_pool(name="sm32", bufs=2))
        tiny = ctx.enter_context(tc.tile_pool(name="tiny", bufs=2 * NBLK))
        scr = ctx.enter_context(tc.tile_pool(name="scr", bufs=2))
        # PSUM pools -- total reserved must fit 8 banks of [128,512]f32
        pbig = ctx.enter_context(tc.tile_pool(name="pbig", bufs=4, space="PSUM"))
        psml = ctx.enter_context(tc.tile_pool(name="psml", bufs=2, space="PSUM"))
        psco = ctx.enter_context(tc.tile_pool(name="psco", bufs=1, space="PSUM"))
        po = ctx.enter_context(tc.tile_pool(name="po", bufs=1, space="PSUM"))

        # ---- constants ------------------------------------------------
        ident = cs.tile([128, 128], F32, tag="ident")
        make_identity(nc, ident[:])
        onesD = cs.tile([128, 128], F32, tag="onesD")   # 1/D for mean matmul
        nc.gpsimd.memset(onesD[:], 1.0 / D)
        ones_row = cs.tile([1, 128], F32, tag="ones_row")
        nc.gpsimd.memset(ones_row[:], 1.0)
        eps1 = cs.tile([1, 1], F32, tag="eps1")
        nc.gpsimd.memset(eps1[:], 1e-5)
        bc2t = cs.tile([1, 1], F32, tag="bc2t")
        nc.gpsimd.memset(bc2t[:], bc2_val)

        def ld(name, dst, src):
            nc.sync.dma_start(out=dst, in_=src)

        kk_sb = [cs.tile([128, KSIZE], F32, tag=f"kk{t}", name=f"kk{t}") for t in range(NDT)]
        for t in range(NDT):
            ld("kk", kk_sb[t][:], kk_d[t * 128:(t + 1) * 128, :])
        wp1_sb = [cs.tile([128, D], F32, tag=f"wp1_{t}", name=f"wp1_{t}") for t in range(NDT)]
        wvg_sb = [cs.tile([128, D], F32, tag=f"wvg_{t}", name=f"wvg_{t}") for t in range(NDT)]
        wqk_sb = [cs.tile([128, HN], F32, tag=f"wqk_{t}", name=f"wqk_{t}") for t in range(NDT)]
        wc2_sb = [cs.tile([128, 1], F32, tag=f"wc2_{t}", name=f"wc2_{t}") for t in range(NDT)]
        bp1_sb = [cs.tile([128, 1], F32, tag=f"bp1_{t}", name=f"bp1_{t}") for t in range(NDT)]
        bpr_sb = [cs.tile([128, 1], F32, tag=f"bpr_{t}", name=f"bpr_{t}") for t in range(NDT)]
        bc1_sb = [cs.tile([128, 1], F32, tag=f"bc1_{t}", name=f"bc1_{t}") for t in range(NDT)]
        for t in range(NDT):
            sl = slice(t * 128, (t + 1) * 128)
            ld("wp1", wp1_sb[t][:], wp1_d[sl, :])
            ld("wvg", wvg_sb[t][:], wvg_d[sl, :])
            ld("wqk", wqk_sb[t][:], wqk_d[sl, :])
            ld("wc2", wc2_sb[t][:], wc2_d[sl, :])
            ld("bp1", bp1_sb[t][:], bp1_d[sl, :])
            ld("bpr", bpr_sb[t][:], bpr_d[sl, :])
            ld("bc1", bc1_sb[t][:], bc1_d[sl, :])
        qwn_sb = cs.tile([HN, 1], F32, tag="qwn")
        qbl_sb = cs.tile([HN, 1], F32, tag="qbl")
        wsv_sb = cs.tile([HN, D], F32, tag="wsv")
        bv_sb = cs.tile([HN, D], F32, tag="bv")
        ld("qwn", qwn_sb[:], qwn_d)
        ld("qbl", qbl_sb[:], qbl_d)
        ld("wsv", wsv_sb[:], wsv_d)
        ld("bv", bv_sb[:], bv_d)
        wj_sb = {}
        for j in pe_taps:
            wj_sb[j] = [cs.tile([128, D], F32, tag=f"wj{j}_{t}", name=f"wj{j}_{t}")
                        for t in range(NDT)]
            for t in range(NDT):
                ld("wj", wj_sb[j][t][:], wj_d[j][t * 128:(t + 1) * 128, :])

        oT_sb = [cs.tile([128, 2 * bc], F32, tag=f"oT_{t}", name=f"oT_{t}") for t in range(NDT)]

        for b in range(bc):
            # ---- load + transpose x -> xT_pad [D, 9+s+9] ---------------
            xT = [xtp.tile([128, s + 2 * HALF], F32, tag=f"xT{t}", name=f"xT{t}")
                  for t in range(NDT)]
            for t in range(NDT):
                nc.gpsimd.memset(xT[t][:, 0:HALF], 0.0)
                nc.gpsimd.memset(xT[t][:, HALF + s:], 0.0)
            for g in range(nst // 4):            # groups of 4 s-tiles
                x_sb = []
                for st in range(4 * g, 4 * g + 4):
                    xt_ = xin.tile([128, D], F32, tag="x_in")
                    nc.sync.dma_start(
                        out=xt_[:], in_=x_d[b, st * 128:(st + 1) * 128, :])
                    x_sb.append(xt_)
                for t in range(NDT):
                    xp = pbig.tile([128, SB], F32, tag="pbig")
                    for i in range(4):
                        nc.tensor.transpose(
                            xp[:, i * 128:(i + 1) * 128],
                            x_sb[i][:, t * 128:(t + 1) * 128], ident[:])
                    nc.scalar.copy(
                        xT[t][:, HALF + g * SB:HALF + (g + 1) * SB], xp[:])

            oP = po.tile([HN, D], F32, tag="oP")
            amb_l, asum_l = [], []

            for blk in range(nblk):
                s0 = blk * SB

                # ---- depthwise conv: acc[t] = sum_j K[:,j]*xT[t, s0+j:+SB]
                acc = [accp.tile([128, SB], F32, tag=f"acc{t}", name=f"acc{t}")
                       for t in range(NDT)]
                for t in range(NDT):
                    j0 = dve_taps[0]
                    nc.vector.tensor_scalar(
                        out=acc[t][:], in0=xT[t][:, s0 + j0:s0 + j0 + SB],
                        scalar1=kk_sb[t][:, j0:j0 + 1], scalar2=None,
                        op0=OP.mult)
                    for j in dve_taps[1:]:
                        nc.vector.scalar_tensor_tensor(
                            out=acc[t][:], in0=xT[t][:, s0 + j:s0 + j + SB],
                            scalar=kk_sb[t][:, j:j + 1], in1=acc[t][:],
                            op0=OP.mult, op1=OP.add)
                    if ag_taps:
                        accg = scr.tile([128, SB], F32, tag="accg")
                        tj = []
                        for j in ag_taps:
                            tj_t = scr.tile([128, SB], F32, tag="tj")
                            nc.scalar.activation(
                                out=tj_t[:], in_=xT[t][:, s0 + j:s0 + j + SB],
                                func=AF.Copy, scale=kk_sb[t][:, j:j + 1])
                            tj.append(tj_t)
                        if len(tj) == 1:
                            nc.vector.tensor_add(acc[t][:], acc[t][:], tj[0][:])
                        else:
                            nc.gpsimd.tensor_add(accg[:], tj[0][:], tj[1][:])
                            for tj_t in tj[2:]:
                                nc.gpsimd.tensor_add(accg[:], accg[:], tj_t[:])
                            nc.vector.tensor_add(acc[t][:], acc[t][:], accg[:])

                # ---- yT = Wp1^T @ posT + xT (+ pe taps) in PSUM --------
                y1p = [pbig.tile([128, SB], F32, tag="pbig", name="y1p")
                       for _ in range(NDT)]
                for et in range(NDT):
                    esl = slice(et * 128, (et + 1) * 128)
                    nc.tensor.matmul(y1p[et][:], wp1_sb[0][:, esl], acc[0][:],
                                     start=True, stop=False)
                    for kt in range(1, NDT):
                        nc.tensor.matmul(y1p[et][:], wp1_sb[kt][:, esl],
                                         acc[kt][:], start=False, stop=False)
                    for j in pe_taps:
                        for kt in range(NDT):
                            nc.tensor.matmul(
                                y1p[et][:], wj_sb[j][kt][:, esl],
                                xT[kt][:, s0 + j:s0 + j + SB],
                                start=False, stop=False)
                    nc.tensor.matmul(
                        y1p[et][:], ident[:],
                        xT[et][:, s0 + HALF:s0 + HALF + SB],
                        start=False, stop=True)

                # ---- drain y (+bias bp1), square --------------------------
                yT, yS = [], []
                for et in range(NDT):
                    yt_ = ydr.tile([128, SB], F32, tag=f"yT{et}")
                    ys_ = ysq.tile([128, SB], F32, tag=f"yS{et}")
                    nc.scalar.activation(out=yt_[:], in_=y1p[et][:],
                                         func=AF.Identity,
                                         bias=bp1_sb[et][:], scale=1.0)
                    nc.scalar.activation(out=ys_[:], in_=yt_[:], func=AF.Square)
                    yT.append(yt_); yS.append(ys_)

                # ---- LN stats (broadcast rows via ones matmul) -----------
                mb = psml.tile([128, SB], F32, tag="psml")
                sq = psml.tile([128, SB], F32, tag="psml")
                for kt in range(NDT):
                    nc.tensor.matmul(mb[:], onesD[:], yT[kt][:],
                                     start=(kt == 0), stop=(kt == NDT - 1))
                for kt in range(NDT):
                    nc.tensor.matmul(sq[:], onesD[:], yS[kt][:],
                                     start=(kt == 0), stop=(kt == NDT - 1))
                m2 = rows.tile([1, SB], F32, tag="rows", name="m2")
                nc.scalar.activation(out=m2[:], in_=mb[0:1, :], func=AF.Square)
                var = rows.tile([1, SB], F32, tag="rows", name="var")
                nc.vector.tensor_sub(var[:], sq[0:1, :], m2[:])
                lnv = rows.tile([1, SB], F32, tag="rows", name="lnv")
                nc.scalar.activation(out=lnv[:], in_=var[:], func=AF.Ln,
                                     bias=eps1[:])
                r_row = rows.tile([1, SB], F32, tag="rows", name="r_row")
                nc.scalar.activation(out=r_row[:], in_=lnv[:], func=AF.Exp,
                                     scale=-0.5)
                rm_row = rows.tile([1, SB], F32, tag="rows", name="rm_row")
                nc.vector.tensor_mul(rm_row[:], r_row[:], mb[0:1, :])

                rbp = psml.tile([HN, SB], F32, tag="psml")
                rmbp = psml.tile([HN, SB], F32, tag="psml")
                nc.tensor.matmul(rbp[:], ones_row[:, 0:HN], r_row[:],
                                 start=True, stop=True)
                nc.tensor.matmul(rmbp[:], ones_row[:, 0:HN], rm_row[:],
                                 start=True, stop=True)
                rb32 = sm32.tile([HN, SB], F32, tag="rb32")
                rmb32 = sm32.tile([HN, SB], F32, tag="rmb32")
                nc.scalar.copy(rb32[:], rbp[:])
                nc.scalar.copy(rmb32[:], rmbp[:])

                # ---- scores + sigmoid + attn out -------------------------
                srw = psco.tile([128, SB], F32, tag="psco")
                for kt in range(NDT):
                    nc.tensor.matmul(srw[0:HN, :], wqk_sb[kt][:], yT[kt][:],
                                     start=(kt == 0), stop=(kt == NDT - 1))
                tsc = sm32.tile([HN, SB], F32, tag="s32scr", name="tsc", bufs=4)
                nc.vector.tensor_mul(tsc[:], srw[0:HN, :], rb32[:])
                sfx = sm32.tile([HN, SB], F32, tag="s32scr", name="sfx", bufs=4)
                nc.vector.scalar_tensor_tensor(
                    out=sfx[:], in0=rmb32[:], scalar=qwn_sb[:], in1=tsc[:],
                    op0=OP.mult, op1=OP.add)
                attn = sm32.tile([HN, SB], F32, tag="attn")
                asum = tiny.tile([HN, 1], F32, tag="asum")
                nc.scalar.activation(out=attn[:], in_=sfx[:], func=AF.Sigmoid,
                                     bias=qbl_sb[:], scale=1.0,
                                     accum_out=asum[:])
                asum_l.append(asum)
                nc.sync.dma_start(out=attn_d[b, :, s0:s0 + SB], in_=attn[:])

                attnR = sm32.tile([HN, SB], F32, tag="attnR")
                nc.vector.tensor_mul(attnR[:], attn[:], rb32[:])
                junk = sm32.tile([HN, SB], F32, tag="s32scr", name="junk", bufs=4)
                amb = tiny.tile([HN, 1], F32, tag="amb")
                nc.vector.scalar_tensor_tensor(
                    out=junk[:], in0=attn[:], scalar=1.0, in1=rmb32[:],
                    op0=OP.mult, op1=OP.mult, accum_out=amb[:])
                amb_l.append(amb)

                # ---- transpose attnR -> [s, HN] tiles --------------------
                atp = psco.tile([128, 128], F32, tag="psco")
                for st in range(4):
                    nc.tensor.transpose(
                        atp[:, st * HN:(st + 1) * HN],
                        attnR[:, st * 128:(st + 1) * 128], ident[0:HN, 0:HN])
                aT = sm32.tile([128, 128], F32, tag="aT")
                nc.scalar.copy(aT[:], atp[:])

                # ---- v = yT^T @ Wvg ; o += attnR^T @ v -------------------
                for st in range(4):
                    vp = pbig.tile([128, D], F32, tag="pbig")
                    ssl = slice(st * 128, (st + 1) * 128)
                    for kt in range(NDT):
                        nc.tensor.matmul(vp[:], yT[kt][:, ssl], wvg_sb[kt][:],
                                         start=(kt == 0), stop=(kt == NDT - 1))
                    vsb = vdr.tile([128, D], F32, tag=f"v{st}")
                    if V_DRAIN == "act":
                        nc.scalar.copy(vsb[:], vp[:])
                    else:
                        nc.vector.tensor_copy(vsb[:], vp[:])
                    nc.tensor.matmul(oP[:], aT[:, st * HN:(st + 1) * HN],
                                     vsb[:],
                                     start=(blk == 0 and st == 0),
                                     stop=(blk == nblk - 1 and st == 3))

            # ---- o fixups, transpose into oT ---------------------------
            amb_t = tiny.tile([HN, 1], F32, tag="amb_t")
            nc.vector.tensor_add(amb_t[:], amb_l[0][:], amb_l[1][:])
            for a in amb_l[2:]:
                nc.vector.tensor_add(amb_t[:], amb_t[:], a[:])
            ambn = tiny.tile([HN, 1], F32, tag="ambn")
            nc.vector.tensor_scalar(out=ambn[:], in0=amb_t[:], scalar1=-1.0,
                                    scalar2=None, op0=OP.mult)
            asum_t = tiny.tile([HN, 1], F32, tag="asum_t")
            nc.vector.tensor_add(asum_t[:], asum_l[0][:], asum_l[1][:])
            for a in asum_l[2:]:
                nc.vector.tensor_add(asum_t[:], asum_t[:], a[:])
            of1 = sm32.tile([HN, D], F32, tag="s32scr", name="of1", bufs=4)
            nc.vector.scalar_tensor_tensor(
                out=of1[:], in0=wsv_sb[:], scalar=ambn[:], in1=oP[:],
                op0=OP.mult, op1=OP.add)
            ofx = sm32.tile([HN, D], F32, tag="ofx")
            nc.vector.scalar_tensor_tensor(
                out=ofx[:], in0=bv_sb[:], scalar=asum_t[:], in1=of1[:],
                op0=OP.mult, op1=OP.add)
            for i in range(NDT):
                otp = psco.tile([128, 128], F32, tag="psco")
                nc.tensor.transpose(otp[:, 0:HN],
                                    ofx[:, i * 128:(i + 1) * 128],
                                    ident[0:HN, 0:HN])
                for hh in range(4):
                    h = 4 * i + hh
                    nc.vector.tensor_copy(
                        oT_sb[i][hh * 32:(hh + 1) * 32, 2 * b:2 * b + 2],
                        otp[hh * 32:(hh + 1) * 32, 2 * h:2 * h + 2])

        # ---- head (once per core, [D, 2*bc] activations) ----------------
        # head weights: loaded late into the recycled wp1/wvg slots
        wpr_sb = [cs.tile([128, D], F32, tag=f"wp1_{t}", name=f"wpr_{t}")
                  for t in range(NDT)]
        wc1_sb = [cs.tile([128, D], F32, tag=f"wvg_{t}", name=f"wc1_{t}")
                  for t in range(NDT)]
        for t in range(NDT):
            sl = slice(t * 128, (t + 1) * 128)
            ld("wpr", wpr_sb[t][:], wpr_d[sl, :])
            ld("wc1", wc1_sb[t][:], wc1_d[sl, :])
        nb = 2 * bc
        hp = []
        for et in range(NDT):
            hpp = pbig.tile([128, nb], F32, tag="pbig")
            esl = slice(et * 128, (et + 1) * 128)
            for kt in range(NDT):
                nc.tensor.matmul(hpp[:], wpr_sb[kt][:, esl], oT_sb[kt][:],
                                 start=(kt == 0), stop=(kt == NDT - 1))
            hsb = sm32.tile([128, nb], F32, tag="headt", name=f"hp{et}", bufs=16)
            nc.scalar.activation(out=hsb[:], in_=hpp[:], func=AF.Identity,
                                 bias=bpr_sb[et][:], scale=1.0)
            hp.append(hsb)
        hq = []
        for et in range(NDT):
            hq_ = sm32.tile([128, nb], F32, tag="headt", name=f"hq{et}", bufs=16)
            nc.scalar.activation(out=hq_[:], in_=hp[et][:], func=AF.Square)
            hq.append(hq_)
        mh = psml.tile([1, nb], F32, tag="psml")
        sh = psml.tile([1, nb], F32, tag="psml")
        for kt in range(NDT):
            nc.tensor.matmul(mh[:], onesD[:, 0:1], hp[kt][:],
                             start=(kt == 0), stop=(kt == NDT - 1))
        for kt in range(NDT):
            nc.tensor.matmul(sh[:], onesD[:, 0:1], hq[kt][:],
                             start=(kt == 0), stop=(kt == NDT - 1))
        m2h = rows.tile([1, nb], F32, tag="rows", name="m2h")
        nc.scalar.activation(out=m2h[:], in_=mh[:], func=AF.Square)
        varh = rows.tile([1, nb], F32, tag="rows", name="varh")
        nc.vector.tensor_sub(varh[:], sh[:], m2h[:])
        lnh = rows.tile([1, nb], F32, tag="rows", name="lnh")
        nc.scalar.activation(out=lnh[:], in_=varh[:], func=AF.Ln, bias=eps1[:])
        rh = rows.tile([1, nb], F32, tag="rows", name="rh")
        nc.scalar.activation(out=rh[:], in_=lnh[:], func=AF.Exp, scale=-0.5)
        m_sb = rows.tile([1, nb], F32, tag="rows", name="m_sb")
        nc.scalar.copy(m_sb[:], mh[0:1, :])
        rbh = psml.tile([128, nb], F32, tag="psml")
        mbh = psml.tile([128, nb], F32, tag="psml")
        nc.tensor.matmul(rbh[:], ones_row[:], rh[:], start=True, stop=True)
        nc.tensor.matmul(mbh[:], ones_row[:], m_sb[:], start=True, stop=True)
        z2 = []
        for et in range(NDT):
            za = sm32.tile([128, nb], F32, tag="headt", name=f"za{et}", bufs=16)
            nc.vector.scalar_tensor_tensor(
                out=za[:], in0=mbh[:], scalar=-1.0, in1=hp[et][:],
                op0=OP.mult, op1=OP.add)
            zb = sm32.tile([128, nb], F32, tag="headt", name=f"zb{et}", bufs=16)
            nc.vector.tensor_mul(zb[:], za[:], rbh[:])
            z2.append(zb)
        h1 = []
        for ct in range(NDT):
            h1p = pbig.tile([128, nb], F32, tag="pbig")
            csl = slice(ct * 128, (ct + 1) * 128)
            for kt in range(NDT):
                nc.tensor.matmul(h1p[:], wc1_sb[kt][:, csl], z2[kt][:],
                                 start=(kt == 0), stop=(kt == NDT - 1))
            h1s = sm32.tile([128, nb], F32, tag="headt", name=f"h1{ct}", bufs=16)
            nc.scalar.activation(out=h1s[:], in_=h1p[:], func=AF.Relu,
                                 bias=bc1_sb[ct][:], scale=1.0)
            h1.append(h1s)
        lgp = psco.tile([1, nb], F32, tag="psco")
        for ct in range(NDT):
            nc.tensor.matmul(lgp[:], wc2_sb[ct][:], h1[ct][:],
                             start=(ct == 0), stop=(ct == NDT - 1))
        lgs = tiny.tile([1, nb], F32, tag="lgs")
        nc.scalar.activation(out=lgs[:], in_=lgp[:], func=AF.Identity,
                             bias=bc2t[:], scale=1.0)
        nc.sync.dma_start(out=log_d[:, :], in_=lgs[:])

    nc.compile()
    return nc


_CACHE = {}


def kernel(**inputs):
    inputs = {k: np.asarray(v) for k, v in inputs.items()}
    c = host_consts(inputs)
    key = "full"
    if key not in _CACHE:
        _CACHE[key] = build(bc2_val=c["bc2"])
    nc = _CACHE[key]
    x = np.ascontiguousarray(inputs["x"], dtype=np.float32)
    in_maps = []
    for i in range(N_CORES):
        m = {"x": np.ascontiguousarray(x[i * BC:(i + 1) * BC])}
        for k, v in c.items():
            if k == "bc2":
                continue
            m[k] = v
        in_maps.append(m)
    res = run_bass_kernel_spmd(nc, in_maps, list(range(N_CORES))).results
    logits = np.concatenate(
        [res[i]["logits"].reshape(BC, NCLS) for i in range(N_CORES)], axis=0)
    attn = np.concatenate(
        [res[i]["attn"].reshape(BC, H, NCLS, S) for i in range(N_CORES)],
        axis=0)
    return logits, attn
